# revision 12
# baseline (speedup 1.0000x reference)
"""HGT (2-type, 2-relation, 2-layer) Bass kernel for 8 Trainium2 cores — v2.

Sharding: destination-node sharding; core c owns dst rows [5120c, 5120(c+1))
of both node types. bf16 on-chip pipeline with fp32 PSUM accumulation.

Key structure vs v1:
- Own-shard K/V projection only; full K/V tables assembled via AllGather of
  bf16 shards (per source-type per layer). K bias dropped (cancels in the
  per-dst softmax); V bias applied after normalization, before gelu.
- Per-edge gather of combined K|V rows (512B bf16) with int16 indices,
  lo/hi split at 32768.
- One-hot (oh: [edge,dst], ohT: [dst,edge]) blocks precomputed on host,
  streamed from DRAM as bf16; per-128-edge-block matmuls do the q gather
  (lhsT=ohT) and the segment-sum scatter (lhsT=oh) with fp32 PSUM accum.
- Vector work batched 4 blocks per instruction; exp/copies on the scalar
  (ACT) engine; input-proj bias folded into the matmul via a ones row.
"""
import math
import os
import sys

import numpy as np

sys.path.insert(0, "/opt/trn_rl_repo")

import ml_dtypes

BF16 = ml_dtypes.bfloat16

H, D, C, L = 4, 32, 128, 2
INV_SQRT_D = 1.0 / math.sqrt(D)
P = 128
NCORES = 8
SHARD = 5120
NGRP = SHARD // P     # 40
NPAD = NCORES * SHARD # 40960
LO_LIM = 32768
HI_ROWS = NPAD - LO_LIM
CHUNK_BLK = 16        # gather chunk size in 128-edge blocks (multiple of 4)
B = 4                 # vector batch size in blocks

LAST_RESULT = None


def _ceil4(x):
    return (x + 3) // 4 * 4


def _fold_weights(ins):
    """Fold a_rel/m_rel into k/v weights, p_rel/sqrt(D) into q, sigmoid(skip)
    into a_lin. K bias dropped (softmax-invariant); V bias kept separately
    (applied post-normalization). Returns dict of numpy arrays (bf16)."""
    f = {}
    for l in range(L):
        for t in range(2):
            kw = np.asarray(ins["k_w"][l, t], np.float32)   # [C, C]
            kb = np.asarray(ins["k_b"][l, t], np.float32)
            vw = np.asarray(ins["v_w"][l, t], np.float32)
            vb = np.asarray(ins["v_b"][l, t], np.float32)
            ar = np.asarray(ins["a_rel"][l, t], np.float32)  # [H, D, D]
            mr = np.asarray(ins["m_rel"][l, t], np.float32)
            wk = np.zeros((C, C), np.float32)
            wv = np.zeros((C, C), np.float32)
            bv = np.zeros(C, np.float32)
            for h in range(H):
                sl = slice(h * D, (h + 1) * D)
                wk[:, sl] = kw[:, sl] @ ar[h]
                wv[:, sl] = vw[:, sl] @ mr[h]
                bv[sl] = vb[sl] @ mr[h]
            del kb
            f[f"Wkv{l}{t}"] = np.concatenate([wk, wv], axis=1).astype(BF16)  # [C,2C]
            # relation t's dst type is 1-t: bv applied in alin(1-t, l)
            f[f"Bv{l}{1 - t}"] = np.tile(bv[None, :], (P, 1)).astype(BF16)
            r_dst = 1 - t
            pr = np.asarray(ins["p_rel"][l, r_dst], np.float32) * INV_SQRT_D
            scale = np.repeat(pr, D)
            f[f"Wq{l}{t}"] = (np.asarray(ins["q_w"][l, t], np.float32) * scale[None, :]).astype(BF16)
            f[f"Wkvq{l}{t}"] = np.concatenate(
                [np.asarray(f[f"Wkv{l}{t}"], np.float32),
                 np.asarray(f[f"Wq{l}{t}"], np.float32)], axis=1).astype(BF16)  # [C,3C]
            bqr = (np.asarray(ins["q_b"][l, t], np.float32) * scale)[None, :]
            f[f"Bkvqr{l}{t}"] = np.concatenate(
                [np.zeros((1, 2 * C), np.float32), bqr], axis=1).astype(BF16)  # [1,3C]
            s = 1.0 / (1.0 + math.exp(-float(np.asarray(ins["skip"][l, t]))))
            f[f"Wal{l}{t}"] = (np.asarray(ins["a_lin_w"][l, t], np.float32) * s).astype(BF16)
            f[f"Balr{l}{t}"] = ((np.asarray(ins["a_lin_b"][l, t], np.float32) * s)[None, :]).astype(BF16)
            f[f"sI{l}{t}"] = ((1.0 - s) * np.eye(P, dtype=np.float32)).astype(BF16)
            f[f"oms{l}{t}"] = 1.0 - s
    # input linears with folded bias row (ones appended to lhsT on host)
    wina = np.asarray(ins["lin_a_w"], np.float32)   # [64, C]
    bina = np.asarray(ins["lin_a_b"], np.float32)
    winb = np.asarray(ins["lin_b_w"], np.float32)   # [32, C]
    binb = np.asarray(ins["lin_b_b"], np.float32)
    f["Wina"] = np.concatenate([wina, bina[None, :]], 0).astype(BF16)  # [65, C]
    f["Winb"] = np.concatenate([winb, binb[None, :]], 0).astype(BF16)  # [33, C]
    return f


def _prep_edges(edge):
    """Partition one relation's edges by dst shard. Returns
    (idx_w[8], oh[8], ohT[8], sched) where sched describes the shared static
    block schedule: dict with nlo, nhi, TLp, THp, TB, and per-block
    (group, first, last) info per region."""
    src = np.asarray(edge[0]).astype(np.int64)
    dst = np.asarray(edge[1]).astype(np.int64)
    core = dst // SHARD
    nlo = np.zeros(NGRP, np.int64)
    nhi = np.zeros(NGRP, np.int64)
    percore = []
    for c in range(NCORES):
        m = core == c
        s, dl = src[m], dst[m] - c * SHARD
        g = dl // P
        rel = dl % P
        lo = s < LO_LIM
        percore.append((s, g, rel, lo))
        for gi in range(NGRP):
            gm = g == gi
            nlo[gi] = max(nlo[gi], int(np.sum(gm & lo)))
            nhi[gi] = max(nhi[gi], int(np.sum(gm & ~lo)))
    nlo = np.maximum((nlo + P - 1) // P, 1)           # blocks per group, >=1
    nhi = (nhi + P - 1) // P
    TL, TH = int(nlo.sum()), int(nhi.sum())
    TLp, THp = _ceil4(TL), _ceil4(TH)
    TB = TLp + THp
    lo_off = np.concatenate([[0], np.cumsum(nlo)[:-1]])
    hi_off = np.concatenate([[0], np.cumsum(nhi)[:-1]]) + TLp

    # per-block group assignment (pads attach to last group)
    blk_grp = np.zeros(TB, np.int64)
    for gi in range(NGRP):
        blk_grp[lo_off[gi]:lo_off[gi] + nlo[gi]] = gi
        blk_grp[hi_off[gi]:hi_off[gi] + nhi[gi]] = gi
    blk_grp[TL:TLp] = NGRP - 1
    blk_grp[TLp + TH:TB] = NGRP - 1

    idx_ws, ohs, ohTs = [], [], []
    for c in range(NCORES):
        s, g, rel, lo = percore[c]
        idx = np.zeros(TB * P, np.int16)
        dr = np.full(TB * P, -1.0, np.float32)
        for gi in range(NGRP):
            for reg, off in ((True, lo_off[gi]), (False, hi_off[gi])):
                gm = (g == gi) & (lo == reg)
                sg, rg = s[gm], rel[gm]
                o = np.argsort(sg, kind="stable")
                sg, rg = sg[o], rg[o]
                base = int(off) * P
                idx[base:base + len(sg)] = (sg if reg else sg - LO_LIM).astype(np.int16)
                dr[base:base + len(sg)] = rg
        idx_ws.append(np.tile(idx.reshape(TB * P // 16, 16).T, (8, 1)).copy())
        drb = dr.reshape(TB, P)                                   # [blk, e]
        j = np.arange(P, dtype=np.float32)
        oh = (drb[:, :, None] == j[None, None, :])                # [blk, e, j]
        ohs.append(np.ascontiguousarray(
            oh.transpose(1, 0, 2).reshape(P, TB * P)).astype(BF16))
        ohT = (drb[:, None, :] == j[None, :, None])               # [blk, j, e]
        ohTs.append(np.ascontiguousarray(
            ohT.transpose(1, 0, 2).reshape(P, TB * P)).astype(BF16))
    sched = dict(nlo=nlo.tolist(), nhi=nhi.tolist(),
                 lo_off=lo_off.tolist(), hi_off=hi_off.tolist(),
                 TL=TL, TH=TH, TLp=TLp, THp=THp, TB=TB,
                 blk_grp=blk_grp.tolist())
    return idx_ws, ohs, ohTs, sched


def kernel(**ins):
    global LAST_RESULT
    import concourse.bass as bass
    import concourse.tile as tile
    from concourse import bacc, mybir
    from concourse.bass_utils import run_bass_kernel_spmd

    FP = mybir.dt.float32
    BF = mybir.dt.bfloat16
    I16 = mybir.dt.int16
    AL = mybir.AluOpType
    AF = mybir.ActivationFunctionType

    f = _fold_weights(ins)
    idx0, oh0, ohT0, sc0 = _prep_edges(np.asarray(ins["edge_ab"]))
    idx1, oh1, ohT1, sc1 = _prep_edges(np.asarray(ins["edge_ba"]))
    scheds = [sc0, sc1]
    TBs = [sc0["TB"], sc1["TB"]]

    xa = np.asarray(ins["x_a"], np.float32)
    xb = np.asarray(ins["x_b"], np.float32)
    DA, DB = xa.shape[1], xb.shape[1]
    # transposed, zero-padded, ones row appended (bias fold), bf16
    xaT = np.zeros((DA + 1, NPAD), np.float32)
    xaT[:DA, :40000] = xa.T
    xaT[DA, :] = 1.0
    xbT = np.zeros((DB + 1, NPAD), np.float32)
    xbT[:DB, :40000] = xb.T
    xbT[DB, :] = 1.0
    xaT = xaT.astype(BF16)
    xbT = xbT.astype(BF16)

    nc = bacc.Bacc("TRN2", target_bir_lowering=False, debug=False, num_devices=NCORES)

    # ---- DRAM tensors ----
    t_xasT = nc.dram_tensor("xasT", [DA + 1, SHARD], BF, kind="ExternalInput").ap()
    t_xbsT = nc.dram_tensor("xbsT", [DB + 1, SHARD], BF, kind="ExternalInput").ap()
    wnames = ["Wina", "Winb"]
    for l in range(L):
        for t in range(2):
            wnames += [f"Wkvq{l}{t}", f"Bkvqr{l}{t}", f"Wal{l}{t}",
                       f"Balr{l}{t}", f"sI{l}{t}", f"Bv{l}{t}"]
    t_w = {n: nc.dram_tensor(n, list(f[n].shape), BF, kind="ExternalInput").ap()
           for n in wnames}
    t_idx = [nc.dram_tensor(f"idx{r}", [P, TBs[r] * 8], I16, kind="ExternalInput").ap()
             for r in range(2)]
    t_oh = [nc.dram_tensor(f"oh{r}", [P, TBs[r] * P], BF, kind="ExternalInput").ap()
            for r in range(2)]
    t_ohT = [nc.dram_tensor(f"ohT{r}", [P, TBs[r] * P], BF, kind="ExternalInput").ap()
             for r in range(2)]

    # K|V tables per (src type, layer): AllGather output, viewed flat for gathers
    t_tab = [[nc.dram_tensor(f"tab{t}{l}", [NCORES, SHARD, 2 * C], BF,
                             addr_space="Shared") for l in range(L)]
             for t in range(2)]
    t_agsrc = [[nc.dram_tensor(f"agsrc{t}{l}", [SHARD, 2 * C], BF)
                for l in range(L)] for t in range(2)]
    t_out = [nc.dram_tensor(f"out{t}", [SHARD, C], FP, kind="ExternalOutput").ap()
             for t in range(2)]

    with tile.TileContext(nc) as tc:
        cpool_cm = tc.tile_pool(name="const", bufs=1)
        cpool = cpool_cm.__enter__()
        ident = cpool.tile([P, P], BF)
        from concourse.masks import make_identity
        make_identity(nc, ident[:])
        w_sb = {}
        for n in wnames:
            w_sb[n] = cpool.tile(list(f[n].shape), BF, name=n, tag=n)
            nc.sync.dma_start(out=w_sb[n][:], in_=t_w[n][:])
        idx_sb = []
        for r in range(2):
            it = cpool.tile([P, TBs[r] * 8], I16, name=f"idxsb{r}", tag=f"idxsb{r}")
            nc.sync.dma_start(out=it[:], in_=t_idx[r][:])
            idx_sb.append(it)
        ones_row = cpool.tile([1, P], BF)
        nc.vector.memset(ones_row[:], 1.0)
        # persistent per-shard state (x kept transposed: [:, g, :] = x^T of group g)
        q_sb = [cpool.tile([P, NGRP, C], BF, name=f"qsb{t}", tag=f"qsb{t}")
                for t in range(2)]
        xT_sb = [[cpool.tile([P, NGRP, C], BF, name=f"xTsb{t}{l}", tag=f"xTsb{t}{l}")
                  for l in range(2)] for t in range(2)]
        acc_sb = [cpool.tile([P, NGRP, 132], FP, name=f"accsb{t}", tag=f"accsb{t}")
                  for t in range(2)]
        xsT_cm = tc.tile_pool(name="xsT", bufs=1)
        xsT_pool = xsT_cm.__enter__()
        xsT_sb = {}
        for t, (ap_, din) in enumerate([(t_xasT, DA + 1), (t_xbsT, DB + 1)]):
            xt = xsT_pool.tile([din, SHARD], BF, name=f"xsT{t}", tag=f"xsT{t}")
            nc.sync.dma_start(out=xt[:], in_=ap_[:])
            xsT_sb[t] = xt

        def ag(t, l):
            if os.environ.get("SKIP_AG"):
                for k in range(NCORES):
                    nc.sync.dma_start(out=t_tab[t][l].ap()[k, :, :],
                                      in_=t_agsrc[t][l].ap()[:])
            else:
                nc.gpsimd.collective_compute(
                    "AllGather", mybir.AluOpType.bypass,
                    replica_groups=[list(range(NCORES))],
                    ins=[t_agsrc[t][l].ap()[:]], outs=[t_tab[t][l].ap()[:]],
                )

        # ---------- phase 1: layer-0 own-shard projections ----------
        def phase1(t):
            Win = "Wina" if t == 0 else "Winb"
            din = (DA if t == 0 else DB) + 1
            with (
                tc.tile_pool(name=f"p1s{t}", bufs=3) as sp,
                tc.tile_pool(name=f"p1p{t}", bufs=2, space="PSUM") as pp,
            ):
                for g in range(NGRP):
                    ps0 = pp.tile([P, C], FP, space="PSUM", tag="c1")
                    nc.tensor.matmul(out=ps0[:], lhsT=xsT_sb[t][:, g * P:(g + 1) * P],
                                     rhs=w_sb[Win][:], start=True, stop=True)
                    x0r = sp.tile([P, C], BF, tag="x0r")
                    nc.scalar.activation(out=x0r[:], in_=ps0[:], func=AF.Relu)
                    pst = pp.tile([P, P], BF, space="PSUM", tag="pst")
                    nc.tensor.transpose(out=pst[:], in_=x0r[:], identity=ident[:])
                    nc.scalar.activation(out=xT_sb[t][0][:, g, :], in_=pst[:], func=AF.Copy)
                    pkv = pp.tile([P, 3 * C], FP, space="PSUM", tag="c2")
                    nc.tensor.matmul(out=pkv[:], lhsT=xT_sb[t][0][:, g, :],
                                     rhs=w_sb[f"Wkvq0{t}"][:], start=True, stop=False)
                    nc.tensor.matmul(out=pkv[:], lhsT=ones_row[:],
                                     rhs=w_sb[f"Bkvqr0{t}"][:], start=False, stop=True)
                    kvt = sp.tile([P, 2 * C], BF, tag="kvt")
                    nc.scalar.activation(out=kvt[:], in_=pkv[:, 0:2 * C], func=AF.Copy)
                    nc.sync.dma_start(out=t_agsrc[t][0].ap()[g * P:(g + 1) * P, :], in_=kvt[:])
                    nc.scalar.activation(out=q_sb[t][:, g, :], in_=pkv[:, 2 * C:3 * C],
                                         func=AF.Copy)

        # ---------- attention ----------
        gpool = bpool = aps = accp = None

        def attention(r, l, mid_cb=None):
            """relation r: src type r, dst type 1-r; fills acc_sb[1-r]."""
            sc = scheds[r]
            td = 1 - r
            tabflat = t_tab[r][l].ap().rearrange("k n c -> (k n) c")
            qt = q_sb[td]
            idxt = idx_sb[r]
            blk_grp = sc["blk_grp"]
            if True:
                chunk_no = 0
                for region in range(2):
                    r0 = 0 if region == 0 else sc["TLp"]
                    r1 = sc["TLp"] if region == 0 else sc["TB"]
                    nblk_reg = r1 - r0
                    if nblk_reg == 0:
                        continue
                    in_ap = tabflat[0:LO_LIM, :] if region == 0 else tabflat[LO_LIM:NPAD, :]
                    accps = None
                    cur_grp = -1
                    for c0 in range(r0, r1, CHUNK_BLK):
                        chunk_no += 1
                        if chunk_no == 4 and mid_cb is not None:
                            mid_cb()
                        n = min(CHUNK_BLK, r1 - c0)
                        gt = gpool.tile([P, CHUNK_BLK, 2 * C], BF, tag="kvchunk")
                        if os.environ.get("SKIP_GATHER"):
                            nc.vector.memset(gt[:, 0:n, :], 1.0)
                        else:
                            nc.gpsimd.dma_gather(
                                out_ap=gt[:, 0:n, :], in_ap=in_ap,
                                idxs_ap=idxt[:, c0 * 8:(c0 + n) * 8],
                                num_idxs=n * P, num_idxs_reg=n * P,
                                elem_size=2 * C, single_packet=False,
                            )
                        oht_c = gpool.tile([P, CHUNK_BLK, P], BF, tag="ohTchunk")
                        nc.sync.dma_start(
                            out=oht_c[:, 0:n, :].rearrange("p a b -> p (a b)"),
                            in_=t_ohT[r][:, c0 * P:(c0 + n) * P])
                        oh_c = gpool.tile([P, CHUNK_BLK, P], BF, tag="ohchunk")
                        nc.sync.dma_start(
                            out=oh_c[:, 0:n, :].rearrange("p a b -> p (a b)"),
                            in_=t_oh[r][:, c0 * P:(c0 + n) * P])
                        for b0 in range(0, n, B):
                            nb = min(B, n - b0)
                            qg_ps = aps.tile([P, B, C], FP, space="PSUM", tag="qg")
                            for i in range(nb):
                                g = blk_grp[c0 + b0 + i]
                                nc.tensor.matmul(out=qg_ps[:, i, :],
                                                 lhsT=oht_c[:, b0 + i, :],
                                                 rhs=qt[:, g, :], start=True, stop=True)
                            qg = bpool.tile([P, B, C], BF, tag="qg_sb")
                            nc.scalar.activation(out=qg[:, 0:nb, :], in_=qg_ps[:, 0:nb, :],
                                                 func=AF.Copy)
                            lp = bpool.tile([P, B, C], BF, tag="lp")
                            nc.vector.tensor_tensor(out=lp[:, 0:nb, :], in0=qg[:, 0:nb, :],
                                                    in1=gt[:, b0:b0 + nb, 0:C], op=AL.mult)
                            z = bpool.tile([P, B * H], FP, tag="z")
                            nc.vector.tensor_reduce(
                                out=z[:, 0:nb * H],
                                in_=lp[:, 0:nb, :].rearrange("p b (h d) -> p (b h) d", h=H),
                                axis=mybir.AxisListType.X, op=AL.add)
                            ze = bpool.tile([P, B * H], BF, tag="ze")
                            nc.scalar.activation(out=ze[:, 0:nb * H], in_=z[:, 0:nb * H],
                                                 func=AF.Exp)
                            zx = bpool.tile([P, B, C], BF, tag="zx")
                            nc.scalar.activation(
                                out=zx[:, 0:nb, :].rearrange("p b (h d) -> p b h d", h=H),
                                in_=ze[:, 0:nb * H].rearrange("p (b h) -> p b h ()", h=H)
                                    .to_broadcast([P, nb, H, D]),
                                func=AF.Copy)
                            wz = bpool.tile([P, B, 132], BF, tag="wz")
                            nc.vector.tensor_tensor(out=wz[:, 0:nb, 0:C],
                                                    in0=gt[:, b0:b0 + nb, C:2 * C],
                                                    in1=zx[:, 0:nb, :], op=AL.mult)
                            nc.scalar.activation(
                                out=wz[:, 0:nb, C:C + H],
                                in_=ze[:, 0:nb * H].rearrange("p (b h) -> p b h", h=H),
                                func=AF.Copy)
                            for i in range(nb):
                                blk = c0 + b0 + i
                                g = blk_grp[blk]
                                if g != cur_grp:
                                    accps = accp.tile([P, 132], FP, space="PSUM", tag="acc")
                                    cur_grp = g
                                off = sc["lo_off"][g] if region == 0 else sc["hi_off"][g]
                                cnt = sc["nlo"][g] if region == 0 else sc["nhi"][g]
                                end = off + cnt
                                if g == NGRP - 1:
                                    end = r1    # pads attach to last group
                                first = blk == off
                                last = blk == end - 1
                                nc.tensor.matmul(out=accps[:], lhsT=oh_c[:, b0 + i, :],
                                                 rhs=wz[:, i, :], start=first, stop=last)
                                if last:
                                    if region == 0:
                                        nc.scalar.activation(out=acc_sb[td][:, g, :],
                                                             in_=accps[:], func=AF.Copy)
                                    else:
                                        nc.vector.tensor_tensor(
                                            out=acc_sb[td][:, g, :], in0=accps[:],
                                            in1=acc_sb[td][:, g, :], op=AL.add)

        # ---------- alin ----------
        def alin(t, l):
            """a_lin + skip for dst type t, layer l; reads acc_sb[t]. For l=0
            also produces layer-1 q, resident x1, and the layer-1 K|V shard."""
            with (
                tc.tile_pool(name=f"al{t}{l}", bufs=3) as sp,
                tc.tile_pool(name=f"alp{t}{l}", bufs=1, space="PSUM") as pp,
            ):
                for g in range(NGRP):
                    den = sp.tile([P, H], FP, tag="den")
                    nc.vector.tensor_scalar(out=den[:], in0=acc_sb[t][:, g, C:C + H],
                                            scalar1=1e-16, scalar2=None, op0=AL.add)
                    rec = sp.tile([P, H], FP, tag="rec")
                    nc.vector.reciprocal(rec[:], den[:])
                    at = sp.tile([P, C], BF, tag="at")
                    nc.vector.tensor_tensor(
                        out=at[:], in0=acc_sb[t][:, g, 0:C],
                        in1=rec[:].rearrange("p (h o) -> p h o", o=1).to_broadcast([P, H, D]),
                        op=AL.mult)
                    at2 = sp.tile([P, C], BF, tag="at2")
                    nc.vector.tensor_tensor(out=at2[:], in0=at[:], in1=w_sb[f"Bv{l}{t}"][:],
                                            op=AL.add)
                    gl = sp.tile([P, C], BF, tag="gl")
                    nc.scalar.activation(out=gl[:], in_=at2[:], func=AF.Gelu)
                    pst = pp.tile([P, P], BF, space="PSUM", tag="trans")
                    nc.tensor.transpose(out=pst[:], in_=gl[:], identity=ident[:])
                    glT = sp.tile([P, P], BF, tag="glT")
                    nc.scalar.activation(out=glT[:], in_=pst[:], func=AF.Copy)
                    pso = pp.tile([P, C], FP, space="PSUM", tag="c1")
                    nc.tensor.matmul(out=pso[:], lhsT=glT[:], rhs=w_sb[f"Wal{l}{t}"][:],
                                     start=True, stop=False)
                    nc.tensor.matmul(out=pso[:], lhsT=ones_row[:],
                                     rhs=w_sb[f"Balr{l}{t}"][:], start=False, stop=False)
                    nc.tensor.matmul(out=pso[:], lhsT=xT_sb[t][l][:, g, :],
                                     rhs=w_sb[f"sI{l}{t}"][:], start=False, stop=True)
                    if l == 0:
                        nw = sp.tile([P, C], BF, tag="nw")
                        nc.scalar.activation(out=nw[:], in_=pso[:], func=AF.Copy)
                        pst2 = pp.tile([P, P], BF, space="PSUM", tag="trans2")
                        nc.tensor.transpose(out=pst2[:], in_=nw[:], identity=ident[:])
                        nc.scalar.activation(out=xT_sb[t][1][:, g, :], in_=pst2[:],
                                             func=AF.Copy)
                        pkv = pp.tile([P, 3 * C], FP, space="PSUM", tag="c2")
                        nc.tensor.matmul(out=pkv[:], lhsT=xT_sb[t][1][:, g, :],
                                         rhs=w_sb[f"Wkvq1{t}"][:], start=True, stop=False)
                        nc.tensor.matmul(out=pkv[:], lhsT=ones_row[:],
                                         rhs=w_sb[f"Bkvqr1{t}"][:], start=False, stop=True)
                        kvt = sp.tile([P, 2 * C], BF, tag="alkvt")
                        nc.scalar.activation(out=kvt[:], in_=pkv[:, 0:2 * C], func=AF.Copy)
                        nc.sync.dma_start(out=t_agsrc[t][1].ap()[g * P:(g + 1) * P, :],
                                          in_=kvt[:])
                        nc.scalar.activation(out=q_sb[t][:, g, :], in_=pkv[:, 2 * C:3 * C],
                                             func=AF.Copy)
                    else:
                        nw = sp.tile([P, C], FP, tag="nwf")
                        nc.scalar.activation(out=nw[:], in_=pso[:], func=AF.Copy)
                        nc.sync.dma_start(out=t_out[t][g * P:(g + 1) * P, :], in_=nw[:])

        # ---------- schedule ----------
        phase1(0)
        ag(0, 0)
        phase1(1)
        ag(1, 0)
        xsT_cm.__exit__(None, None, None)
        gpool_cm = tc.tile_pool(name="gat", bufs=4)
        gpool = gpool_cm.__enter__()
        bpool_cm = tc.tile_pool(name="bat", bufs=3)
        bpool = bpool_cm.__enter__()
        aps_cm = tc.tile_pool(name="aps", bufs=2, space="PSUM")
        aps = aps_cm.__enter__()
        accp_cm = tc.tile_pool(name="accp", bufs=2, space="PSUM")
        accp = accp_cm.__enter__()
        attention(0, 0)     # dst b, table a/l0
        alin(1, 0)          # x1_b, kv_b_l1 shard
        attention(1, 0, mid_cb=lambda: ag(1, 1))   # dst a, table b/l0
        alin(0, 0)          # x1_a, kv_a_l1 shard
        attention(1, 1, mid_cb=lambda: ag(0, 1))   # dst a, table b/l1
        alin(0, 1)          # out a
        attention(0, 1)     # dst b, table a/l1
        alin(1, 1)          # out b
        accp_cm.__exit__(None, None, None)
        aps_cm.__exit__(None, None, None)
        bpool_cm.__exit__(None, None, None)
        gpool_cm.__exit__(None, None, None)
        cpool_cm.__exit__(None, None, None)

    nc.compile()

    in_maps = []
    for c in range(NCORES):
        m = {"xasT": np.ascontiguousarray(xaT[:, c * SHARD:(c + 1) * SHARD]),
             "xbsT": np.ascontiguousarray(xbT[:, c * SHARD:(c + 1) * SHARD]),
             "idx0": idx0[c], "oh0": oh0[c], "ohT0": ohT0[c],
             "idx1": idx1[c], "oh1": oh1[c], "ohT1": ohT1[c]}
        for n in wnames:
            m[n] = np.ascontiguousarray(f[n])
        in_maps.append(m)

    res = run_bass_kernel_spmd(
        nc, in_maps, core_ids=list(range(NCORES)),
        trace=bool(os.environ.get("BASS_TRACE")),
    )
    LAST_RESULT = res
    outa = np.concatenate([res.results[c]["out0"] for c in range(NCORES)])[:40000]
    outb = np.concatenate([res.results[c]["out1"] for c in range(NCORES)])[:40000]
    return outa, outb
